# revision 16
# baseline (speedup 1.0000x reference)
"""Multi-head attention (RoPE + causal softmax) Trainium2 Bass kernel.

Problem: nn_MultiHeadAttention (B=16, S=512, D=1024, H=16, Hd=64).
Sharding: data-parallel over batch — 2 batches per core on 8 NeuronCores.

Device-side layout is feature-major ("transposed"): activations live as
[d, token] tiles so the d contraction sits on SBUF partitions for every
matmul. Per core:

  xT        [1024, 1024]  bf16   x shard, feature-major (col = b*512 + s)
  WqT/WkT/WvT/WoT [1024, 1024] bf16  (nn.Linear weight, transposed)
  cos2/sin2f [128, 2, 512] bf16  RoPE tables; sin2f has rotate_half's
                                 sign pattern folded in
  mask2     [128, 2, 128] bf16   causal 0/1 mask for diagonal blocks
  outT      [1024, 1024]  bf16   output, feature-major

Pipeline: q+k projections land in one [128,(q|k),512] bf16 tile;
rotate_half is 4 partition-block-swap SBUF->SBUF DMAs; RoPE combine is
3 full-width contiguous bf16 DVE ops (per-(m,batch) qkrot tiles keep
the writes dense). v is token-major with a ones-column per head so
attn@v also yields softmax sums. Per (batch, head-pair): scores^T in
two concurrent 64-row PE groups -> exp (ACT, scale=1/8) -> diag-block
mask -> attn@v into a [128,(h0|h1),512] PSUM tile evacuated by ONE
[65,2,512] bf16 copy (sums row included, PSUM freed fast). The
normalize tail (reshape DMA -> reciprocal -> cast DMA -> partition
broadcast -> two 2x-rate muls) is emitted STAGGERED over later pair
slots so stalled ops never block the in-order engine queues that feed
the PE. Input DMAs are split/interleaved by column halves so the PE
starts within a few us; wo groups are absorbed into the attention
phases. Host reassembles [16, 512, 1024] fp32.
"""

import numpy as np
import ml_dtypes

BF16 = ml_dtypes.bfloat16

B, S, D = 16, 512, 1024
H, HD = 16, 64
NCORES = 8
BPC = B // NCORES          # batches per core
T = BPC * S                # tokens per core

_CACHE = {}


def _rope_tables():
    inv_freq = 1.0 / (10000.0 ** (np.arange(0, HD, 2, dtype=np.float32) / HD))
    t = np.arange(S, dtype=np.float32)
    freqs = np.outer(t, inv_freq)                    # [S, 32]
    emb = np.concatenate([freqs, freqs], -1)         # [S, 64]
    return np.cos(emb), np.sin(emb)                  # [S, 64] fp32


def _host_consts():
    cos, sin = _rope_tables()                        # [S, 64]
    cosd = np.tile(cos.T, (2, 1))                    # [128, S]
    sind = np.tile(sin.T, (2, 1))
    sgn = np.where((np.arange(128) % 64) < 32, -1.0, 1.0)[:, None]
    sinf = sind * sgn
    cos2 = np.ascontiguousarray(np.broadcast_to(cosd[:, None, :], (128, 2, S))).astype(BF16)
    sin2f = np.ascontiguousarray(np.broadcast_to(sinf[:, None, :], (128, 2, S))).astype(BF16)
    m = (np.arange(128)[None, :] >= np.arange(128)[:, None]).astype(np.float32)  # [kt, qt]
    mask2 = np.ascontiguousarray(np.broadcast_to(m[:, None, :], (128, 2, 128))).astype(BF16)
    return cos2, sin2f, mask2


def _build_bass(dump_debug=False):
    import concourse.bacc as bacc
    import concourse.tile as tile
    import concourse.mybir as mybir

    dt = mybir.dt
    f32, bf16 = dt.float32, dt.bfloat16
    Exp = mybir.ActivationFunctionType.Exp

    nc = bacc.Bacc("TRN2", target_bir_lowering=False, debug=False, enable_asserts=False)

    xT_d = nc.dram_tensor("xT", [D, T], bf16, kind="ExternalInput").ap()
    wq_d = nc.dram_tensor("WqT", [D, D], bf16, kind="ExternalInput").ap()
    wk_d = nc.dram_tensor("WkT", [D, D], bf16, kind="ExternalInput").ap()
    wv_d = nc.dram_tensor("WvT", [D, D], bf16, kind="ExternalInput").ap()
    wo_d = nc.dram_tensor("WoT", [D, D], bf16, kind="ExternalInput").ap()
    cos_d = nc.dram_tensor("cos2", [128, 2, S], bf16, kind="ExternalInput").ap()
    sin_d = nc.dram_tensor("sin2f", [128, 2, S], bf16, kind="ExternalInput").ap()
    mask_d = nc.dram_tensor("mask2", [128, 2, 128], bf16, kind="ExternalInput").ap()
    out_d = nc.dram_tensor("outT", [D, T], bf16, kind="ExternalOutput").ap()
    if dump_debug:
        qrot_d = nc.dram_tensor("qrotD", [D, T], bf16, kind="ExternalOutput").ap()
        krot_d = nc.dram_tensor("krotD", [D, T], bf16, kind="ExternalOutput").ap()
        v_d = nc.dram_tensor("vD", [T, H * 65], bf16, kind="ExternalOutput").ap()
        att_d = nc.dram_tensor("attD", [D, T], bf16, kind="ExternalOutput").ap()

    KC = D // 128  # 8 contraction chunks

    with tile.TileContext(nc) as tc:
        with (
            tc.tile_pool(name="consts", bufs=1) as consts,
            tc.tile_pool(name="persist", bufs=1) as persist,
            tc.tile_pool(name="work", bufs=2) as work,
            tc.tile_pool(name="expp", bufs=2) as expp,
            tc.tile_pool(name="ps_p", bufs=2, space="PSUM") as ps_p,
            tc.tile_pool(name="ps_sc", bufs=2, space="PSUM") as ps_sc,
            tc.tile_pool(name="ps_av", bufs=1, space="PSUM") as ps_av,
        ):
            # ---- resident input tiles: one multi-chunk tile per tensor so a
            # single DMA instruction loads all 8 contraction chunks (the sync
            # ring's ~0.6us per-DMA issue cost was gating the kernel head)
            xTa = consts.tile([128, KC, T], bf16, name="xTa")
            wqa = consts.tile([128, KC, D], bf16, name="wqa")
            wka = consts.tile([128, KC, D], bf16, name="wka")
            wva = consts.tile([128, KC, D], bf16, name="wva")
            woa = consts.tile([128, KC, D], bf16, name="woa")
            xT = [xTa[:, k, :] for k in range(KC)]
            wq = [wqa[:, k, :] for k in range(KC)]
            wk = [wka[:, k, :] for k in range(KC)]
            wv = [wva[:, k, :] for k in range(KC)]
            wo = [woa[:, k, :] for k in range(KC)]
            cos2 = consts.tile([128, 2, S], bf16, name="cos2")
            sin2f = consts.tile([128, 2, S], bf16, name="sin2f")
            mask2 = consts.tile([128, 2, 128], bf16, name="mask2")

            xT_r = xT_d.rearrange("(k p) c -> p k c", p=128)
            wq_r = wq_d.rearrange("(k p) c -> p k c", p=128)
            wk_r = wk_d.rearrange("(k p) c -> p k c", p=128)
            wv_r = wv_d.rearrange("(k p) c -> p k c", p=128)
            wo_r = wo_d.rearrange("(k p) c -> p k c", p=128)

            def half_load(t_, dram, h, eng):
                hw = t_.shape[-1] // 2
                eng.dma_start(out=t_[:, :, h * hw:(h + 1) * hw],
                              in_=dram[:, :, h * hw:(h + 1) * hw])

            # priority order: q/k-proj b0 + rope tables + v, then second
            # halves; alternate the two HWDGE rings so two streams run in
            # parallel (a single hardware queue serializes at ~360 GB/s)
            half_load(xTa, xT_r, 0, nc.sync)
            nc.sync.dma_start(out=wqa[:, :, 0:256], in_=wq_r[:, :, 0:256])
            nc.sync.dma_start(out=wqa[:, :, 256:512], in_=wq_r[:, :, 256:512])
            half_load(wka, wk_r, 0, nc.sync)
            nc.scalar.dma_start(out=cos2, in_=cos_d)
            nc.scalar.dma_start(out=sin2f, in_=sin_d)
            nc.scalar.dma_start(out=mask2, in_=mask_d)
            half_load(wva, wv_r, 0, nc.sync)
            half_load(wqa, wq_r, 1, nc.sync)
            half_load(wka, wk_r, 1, nc.sync)
            half_load(xTa, xT_r, 1, nc.sync)
            half_load(wva, wv_r, 1, nc.sync)
            nc.sync.dma_start(out=woa, in_=wo_r)

            # ---- persistent intermediates
            # qkrot[m][b]: [:, 0, :] = qrot, [:, 1, :] = krot (contiguous per batch)
            qkrot = [[persist.tile([128, 2, S], bf16, name=f"qkrot{m}_{b}")
                      for b in range(2)] for m in range(KC)]
            # v token-major, per head padded with a ones column (65 per head)
            vsb = [persist.tile([128, H * 65], bf16, name=f"vsb{t_}") for t_ in range(T // 128)]
            att = [persist.tile([128, T], bf16, name=f"att{m}") for m in range(KC)]

            for t_ in range(T // 128):
                vt = vsb[t_].rearrange("p (h w) -> p h w", w=65)
                nc.gpsimd.memset(vt[:, :, 64:65], 1.0)

            # dummy exp: pull the ACT exp table load into the DMA phase
            dumm = work.tile([1, 8], f32, name="dumm", tag="dumm", bufs=1)
            nc.gpsimd.memset(dumm, 0.0)
            nc.scalar.activation(dumm, dumm, Exp, scale=0.125)

            # ---- phase emitters
            def emit_qk_group(b, m):
                cols = slice(b * S, (b + 1) * S)
                pre2 = work.tile([128, 2, S], bf16, name="pre2", tag="pre2", bufs=2)
                for qk, w_sb in ((0, wq), (1, wk)):
                    pp = ps_p.tile([128, S], f32, name="pp", tag="ps_p")
                    for k in range(KC):
                        nc.tensor.matmul(
                            pp, w_sb[k][:, m * 128:(m + 1) * 128], xT[k][:, cols],
                            start=(k == 0), stop=(k == KC - 1))
                    nc.scalar.copy(pre2[:, qk, :], pp)   # ACT: psum -> sbuf bf16
                # rotate_half: 4 partition-block-swap DMAs (b0 on gpsimd ring,
                # b1 on the sync ring which is idle once loads finish)
                deng = nc.gpsimd if b == 0 else nc.sync
                prot2 = work.tile([128, 2, S], bf16, name="prot2", tag="prot2", bufs=2)
                for blk in range(4):
                    src = slice((blk ^ 1) * 32, (blk ^ 1) * 32 + 32)
                    dst = slice(blk * 32, blk * 32 + 32)
                    deng.dma_start(out=prot2[dst, :, :], in_=pre2[src, :, :])
                t1 = work.tile([128, 2, S], bf16, name="t1", tag="t1", bufs=2)
                nc.vector.tensor_mul(t1, pre2, cos2)
                t2 = work.tile([128, 2, S], bf16, name="t2", tag="t2", bufs=2)
                nc.vector.tensor_mul(t2, prot2, sin2f)
                nc.vector.tensor_add(qkrot[m][b], t1, t2)

            def emit_v_group(b, tch, nh):
                vt = vsb[tch].rearrange("p (h w) -> p h w", w=65)
                vp = ps_p.tile([128, S], f32, name="vp", tag="ps_p")
                for k in range(KC):
                    nc.tensor.matmul(
                        vp, xT[k][:, tch * 128:(tch + 1) * 128],
                        wv[k][:, nh * S:(nh + 1) * S],
                        start=(k == 0), stop=(k == KC - 1))
                nc.scalar.copy(
                    vt[:, nh * 8:(nh + 1) * 8, 0:64],
                    vp.rearrange("p (h w) -> p h w", w=64))

            # attention pair stages; state tiles kept per pair index
            stash = {}

            def attn_stage0(b, j):
                """scores -> exp -> mask -> attn@v -> evacuate -> st reshape"""
                mh = j
                exs = []
                for i in range(4):
                    lo = i * 128
                    sc = ps_sc.tile([128, 2, S], f32, name="sc", tag="ps_sc")
                    for hi, p0 in ((0, 0), (1, 64)):
                        nc.tensor.matmul(
                            sc[:, hi, 0:S - lo],
                            qkrot[mh][b][p0:p0 + 64, 1, lo:lo + 128],
                            qkrot[mh][b][p0:p0 + 64, 0, lo:S],
                            start=True, stop=True)
                    ex = expp.tile([128, 2, S], bf16, name="ex", tag=f"ex{i}")
                    nc.scalar.activation(ex[:, :, lo:S], sc[:, :, 0:S - lo], Exp, scale=0.125)
                    nc.vector.tensor_mul(ex[:, :, lo:lo + 128], ex[:, :, lo:lo + 128], mask2)
                    exs.append(ex)
                av = ps_av.tile([128, 2, S], f32, name="av", tag="ps_av")
                for hi in (0, 1):
                    h = 2 * j + hi
                    for i in range(4):
                        lo = i * 128
                        nc.tensor.matmul(
                            av[0:65, hi, lo:S],
                            vsb[b * 4 + i][:, h * 65: h * 65 + 65],
                            exs[i][:, hi, lo:S],
                            start=(i == 0), stop=(i == 3), skip_group_check=True)
                # single evacuation: raw att (rows 0:64) + sums row (64)
                attss = work.tile([65, 2, S], bf16, name="attss", tag="attss", bufs=3)
                nc.vector.tensor_copy(attss, av[0:65, :, :])
                # reshape sums row so the reciprocal uses all DVE lanes
                st = work.tile([128, 8], bf16, name="st", tag="st", bufs=2)
                nc.sync.dma_start(out=st, in_=attss[64:65, :, :])
                stash[(b, j)] = (attss, st)

            def attn_stage1(b, j):
                """reciprocal -> cast+reshape back -> broadcast to 64 rows"""
                attss, st = stash[(b, j)]
                rt = work.tile([128, 8], bf16, name="rt", tag="rt", bufs=2)
                with nc.allow_low_precision("softmax denominators in bf16"):
                    nc.vector.reciprocal(rt, st)
                rr = work.tile([1, 2 * S], bf16, name="rr", tag="rr", bufs=2)
                nc.gpsimd.dma_start(out=rr, in_=rt)
                rb = work.tile([64, 2 * S], bf16, name="rb", tag="rb", bufs=2)
                nc.gpsimd.partition_broadcast(rb, rr)
                stash[(b, j)] = (attss, rb)

            def attn_stage2(b, j):
                attss, rb = stash.pop((b, j))
                bcols = slice(b * S, (b + 1) * S)
                nc.vector.tensor_mul(att[j][0:64, bcols], attss[0:64, 0, :], rb[:, 0:S])
                nc.vector.tensor_mul(att[j][64:128, bcols], attss[0:64, 1, :], rb[:, S:2 * S])

            def emit_wo_group(b, m, ob_eng):
                bcols = slice(b * S, (b + 1) * S)
                fin = ps_p.tile([128, S], f32, name="fin", tag="ps_p")
                for k in range(KC):
                    nc.tensor.matmul(
                        fin, wo[k][:, m * 128:(m + 1) * 128], att[k][:, bcols],
                        start=(k == 0), stop=(k == KC - 1))
                ob = work.tile([128, S], bf16, name="ob", tag="ob", bufs=2)
                if ob_eng == "scalar":
                    nc.scalar.copy(ob, fin)
                else:
                    nc.vector.tensor_copy(ob, fin)
                nc.sync.dma_start(out=out_d[m * 128:(m + 1) * 128, bcols], in_=ob)

            # ---- emission schedule
            # phase A: b0 projections (ordered to chase the load stream)
            for m in range(4):
                emit_qk_group(0, m)
            for tch in range(4):
                emit_v_group(0, tch, 0)
            for m in range(4, KC):
                emit_qk_group(0, m)
            for tch in range(4):
                emit_v_group(0, tch, 1)
            for tch in range(4, 8):
                emit_v_group(1, tch, 0)
            # phase B: pairs b0 + staggered normalize; fills: qk b1, v b1.
            # NOTE: wo(b, m) reads ALL att chunks of batch b, so wo(0,*) may
            # only be emitted after stage2(0, 7), and wo(1,*) after stage2(1, 7).
            v1 = [(4, 1), (5, 1), (6, 1), (7, 1)]
            for j in range(8):
                attn_stage0(0, j)
                if j < 4:
                    emit_qk_group(1, j)
                if j in (6, 7):
                    emit_qk_group(1, j - 2)
                if 2 <= j < 6:
                    emit_v_group(1, *v1[j - 2])
                if j >= 1:
                    attn_stage1(0, j - 1)
                if j >= 2:
                    attn_stage2(0, j - 2)
            # phase C: pairs b1; fills: remaining qk b1, wo b0
            for j in range(8):
                attn_stage0(1, j)
                if j < 2:
                    emit_qk_group(1, 6 + j)
                if j == 0:
                    attn_stage1(0, 7)
                if j == 1:
                    attn_stage2(0, 6)
                if j == 2:
                    attn_stage2(0, 7)
                if j >= 1 and j < 7:
                    attn_stage1(1, j - 1)
                if j >= 2 and j < 7:
                    attn_stage2(1, j - 2)
                if j >= 3:
                    emit_wo_group(0, j - 3, "vector")
                if j >= 5:
                    emit_wo_group(0, j, "vector")
                if j == 6:
                    attn_stage1(1, 6)
                if j == 7:
                    attn_stage2(1, 5)
                    attn_stage2(1, 6)
                    attn_stage1(1, 7)
            attn_stage2(1, 7)
            # phase D: wo b1 (needs all b1 att chunks)
            for m in range(KC):
                emit_wo_group(1, m, "scalar")

            if dump_debug:
                for m in range(KC):
                    rows = slice(m * 128, (m + 1) * 128)
                    for b in range(2):
                        cl = slice(b * S, (b + 1) * S)
                        nc.sync.dma_start(out=qrot_d[rows, cl], in_=qkrot[m][b][:, 0, :])
                        nc.sync.dma_start(out=krot_d[rows, cl], in_=qkrot[m][b][:, 1, :])
                    nc.sync.dma_start(out=att_d[rows, :], in_=att[m])
                for t_ in range(T // 128):
                    nc.sync.dma_start(out=v_d[t_ * 128:(t_ + 1) * 128, :], in_=vsb[t_])

    nc.compile()
    return nc


def _get_nc():
    if "nc" not in _CACHE:
        _CACHE["nc"] = _build_bass()
    return _CACHE["nc"]


def make_in_maps(x, Wq, Wk, Wv, Wo):
    """Host-side shard + layout prep: one input dict per core."""
    cos2, sin2f, mask2 = _host_consts()
    shared = {
        "WqT": np.ascontiguousarray(Wq.T).astype(BF16),
        "WkT": np.ascontiguousarray(Wk.T).astype(BF16),
        "WvT": np.ascontiguousarray(Wv.T).astype(BF16),
        "WoT": np.ascontiguousarray(Wo.T).astype(BF16),
        "cos2": cos2,
        "sin2f": sin2f,
        "mask2": mask2,
    }
    in_maps = []
    for c in range(NCORES):
        xc = x[c * BPC:(c + 1) * BPC]  # [BPC, S, D]
        xT = np.ascontiguousarray(xc.transpose(2, 0, 1).reshape(D, T)).astype(BF16)
        in_maps.append({"xT": xT, **shared})
    return in_maps


def assemble(results):
    """results: list (per core) of {"outT": [D, T] bf16} -> [B, S, D] fp32."""
    out = np.empty((B, S, D), np.float32)
    for c in range(NCORES):
        oT = np.asarray(results[c]["outT"]).astype(np.float32)
        out[c * BPC:(c + 1) * BPC] = oT.reshape(D, BPC, S).transpose(1, 2, 0)
    return out


def run(x, Wq, Wk, Wv, Wo, trace=False, **run_kwargs):
    from concourse.bass_utils import run_bass_kernel_spmd
    nc = _get_nc()
    in_maps = make_in_maps(x, Wq, Wk, Wv, Wo)
    res = run_bass_kernel_spmd(
        nc, in_maps, core_ids=list(range(NCORES)), trace=trace, **run_kwargs)
    return assemble(res.results), res


def kernel(x, Wq, Wk, Wv, Wo):
    out, _ = run(np.asarray(x), np.asarray(Wq), np.asarray(Wk),
                 np.asarray(Wv), np.asarray(Wo))
    return out


if __name__ == "__main__":
    rng = np.random.default_rng(0)
    scale = 1.0 / np.sqrt(D)
    inputs = {
        "x": rng.standard_normal((B, S, D), dtype=np.float32),
        "Wq": (rng.standard_normal((D, D), dtype=np.float32) * scale),
        "Wk": (rng.standard_normal((D, D), dtype=np.float32) * scale),
        "Wv": (rng.standard_normal((D, D), dtype=np.float32) * scale),
        "Wo": (rng.standard_normal((D, D), dtype=np.float32) * scale),
    }
    out = kernel(**inputs)
    print("out", out.shape, out.dtype, float(np.abs(out).max()))


# revision 18
# speedup vs baseline: 1.0522x; 1.0522x over previous
"""Multi-head attention (RoPE + causal softmax) Trainium2 Bass kernel.

Problem: nn_MultiHeadAttention (B=16, S=512, D=1024, H=16, Hd=64).
Sharding: data-parallel over batch — 2 batches per core on 8 NeuronCores.

Device-side layout is feature-major ("transposed"): activations live as
[d, token] tiles so the d contraction sits on SBUF partitions for every
matmul. Per core:

  xT        [1024, 1024]  bf16   x shard, feature-major (col = b*512 + s)
  WqT/WkT/WvT/WoT [1024, 1024] bf16  (nn.Linear weight, transposed)
  cos2/sin2f [128, 2, 512] bf16  RoPE tables; sin2f has rotate_half's
                                 sign pattern folded in
  mask2     [128, 2, 128] bf16   causal 0/1 mask for diagonal blocks
  outT      [1024, 1024]  bf16   output, feature-major

Pipeline: q+k projections land in one [128,(q|k),512] bf16 tile;
rotate_half is 4 partition-block-swap SBUF->SBUF DMAs; RoPE combine is
3 full-width contiguous bf16 DVE ops (per-(m,batch) qkrot tiles keep
the writes dense). v is token-major with a ones-column per head so
attn@v also yields softmax sums. Per (batch, head-pair): scores^T in
two concurrent 64-row PE groups -> exp (ACT, scale=1/8) -> diag-block
mask -> attn@v into a [128,(h0|h1),512] PSUM tile evacuated by ONE
[65,2,512] bf16 copy (sums row included, PSUM freed fast). The
normalize tail (reshape DMA -> reciprocal -> cast DMA -> partition
broadcast -> two 2x-rate muls) is emitted STAGGERED over later pair
slots so stalled ops never block the in-order engine queues that feed
the PE. Input DMAs are split/interleaved by column halves so the PE
starts within a few us; wo groups are absorbed into the attention
phases. Host reassembles [16, 512, 1024] fp32.
"""

import numpy as np
import ml_dtypes

BF16 = ml_dtypes.bfloat16

B, S, D = 16, 512, 1024
H, HD = 16, 64
NCORES = 8
BPC = B // NCORES          # batches per core
T = BPC * S                # tokens per core

_CACHE = {}


def _rope_tables():
    inv_freq = 1.0 / (10000.0 ** (np.arange(0, HD, 2, dtype=np.float32) / HD))
    t = np.arange(S, dtype=np.float32)
    freqs = np.outer(t, inv_freq)                    # [S, 32]
    emb = np.concatenate([freqs, freqs], -1)         # [S, 64]
    return np.cos(emb), np.sin(emb)                  # [S, 64] fp32


def _host_consts():
    cos, sin = _rope_tables()                        # [S, 64]
    cosd = np.tile(cos.T, (2, 1))                    # [128, S]
    sind = np.tile(sin.T, (2, 1))
    sgn = np.where((np.arange(128) % 64) < 32, -1.0, 1.0)[:, None]
    sinf = sind * sgn
    cos2 = np.ascontiguousarray(np.broadcast_to(cosd[:, None, :], (128, 2, S))).astype(BF16)
    sin2f = np.ascontiguousarray(np.broadcast_to(sinf[:, None, :], (128, 2, S))).astype(BF16)
    m = (np.arange(128)[None, :] >= np.arange(128)[:, None]).astype(np.float32)  # [kt, qt]
    mask2 = np.ascontiguousarray(np.broadcast_to(m[:, None, :], (128, 2, 128))).astype(BF16)
    return cos2, sin2f, mask2


def _build_bass(dump_debug=False):
    import concourse.bacc as bacc
    import concourse.tile as tile
    import concourse.mybir as mybir

    dt = mybir.dt
    f32, bf16 = dt.float32, dt.bfloat16
    Exp = mybir.ActivationFunctionType.Exp

    nc = bacc.Bacc("TRN2", target_bir_lowering=False, debug=False, enable_asserts=False)

    xT_d = nc.dram_tensor("xT", [D, T], bf16, kind="ExternalInput").ap()
    wq_d = nc.dram_tensor("WqT", [D, D], bf16, kind="ExternalInput").ap()
    wk_d = nc.dram_tensor("WkT", [D, D], bf16, kind="ExternalInput").ap()
    wv_d = nc.dram_tensor("WvT", [D, D], bf16, kind="ExternalInput").ap()
    wo_d = nc.dram_tensor("WoT", [D, D], bf16, kind="ExternalInput").ap()
    cos_d = nc.dram_tensor("cos2", [128, 2, S], bf16, kind="ExternalInput").ap()
    sin_d = nc.dram_tensor("sin2f", [128, 2, S], bf16, kind="ExternalInput").ap()
    mask_d = nc.dram_tensor("mask2", [128, 2, 128], bf16, kind="ExternalInput").ap()
    out_d = nc.dram_tensor("outT", [D, T], bf16, kind="ExternalOutput").ap()
    if dump_debug:
        qrot_d = nc.dram_tensor("qrotD", [D, T], bf16, kind="ExternalOutput").ap()
        krot_d = nc.dram_tensor("krotD", [D, T], bf16, kind="ExternalOutput").ap()
        v_d = nc.dram_tensor("vD", [T, H * 65], bf16, kind="ExternalOutput").ap()
        att_d = nc.dram_tensor("attD", [D, T], bf16, kind="ExternalOutput").ap()

    KC = D // 128  # 8 contraction chunks

    with tile.TileContext(nc) as tc:
        with (
            tc.tile_pool(name="consts", bufs=1) as consts,
            tc.tile_pool(name="persist", bufs=1) as persist,
            tc.tile_pool(name="work", bufs=2) as work,
            tc.tile_pool(name="expp", bufs=2) as expp,
            tc.tile_pool(name="ps_p", bufs=2, space="PSUM") as ps_p,
            tc.tile_pool(name="ps_sc", bufs=2, space="PSUM") as ps_sc,
            tc.tile_pool(name="ps_av", bufs=1, space="PSUM") as ps_av,
        ):
            # ---- resident input tiles: one multi-chunk tile per tensor so a
            # single DMA instruction loads all 8 contraction chunks (the sync
            # ring's ~0.6us per-DMA issue cost was gating the kernel head)
            xTa = consts.tile([128, KC, T], bf16, name="xTa")
            wqa = consts.tile([128, KC, D], bf16, name="wqa")
            wka = consts.tile([128, KC, D], bf16, name="wka")
            wva = consts.tile([128, KC, D], bf16, name="wva")
            woa = consts.tile([128, KC, D], bf16, name="woa")
            xT = [xTa[:, k, :] for k in range(KC)]
            wq = [wqa[:, k, :] for k in range(KC)]
            wk = [wka[:, k, :] for k in range(KC)]
            wv = [wva[:, k, :] for k in range(KC)]
            wo = [woa[:, k, :] for k in range(KC)]
            cos2 = consts.tile([128, 2, S], bf16, name="cos2")
            sin2f = consts.tile([128, 2, S], bf16, name="sin2f")
            mask2 = consts.tile([128, 2, 128], bf16, name="mask2")

            xT_r = xT_d.rearrange("(k p) c -> p k c", p=128)
            wq_r = wq_d.rearrange("(k p) c -> p k c", p=128)
            wk_r = wk_d.rearrange("(k p) c -> p k c", p=128)
            wv_r = wv_d.rearrange("(k p) c -> p k c", p=128)
            wo_r = wo_d.rearrange("(k p) c -> p k c", p=128)

            def half_load(t_, dram, h, eng):
                hw = t_.shape[-1] // 2
                eng.dma_start(out=t_[:, :, h * hw:(h + 1) * hw],
                              in_=dram[:, :, h * hw:(h + 1) * hw])

            # priority order: q/k-proj b0 + rope tables + v, then second
            # halves; alternate the two HWDGE rings so two streams run in
            # parallel (a single hardware queue serializes at ~360 GB/s)
            half_load(xTa, xT_r, 0, nc.sync)
            nc.sync.dma_start(out=wqa[:, :, 0:128], in_=wq_r[:, :, 0:128])
            nc.sync.dma_start(out=wqa[:, :, 128:512], in_=wq_r[:, :, 128:512])
            nc.sync.dma_start(out=wka[:, :, 0:128], in_=wk_r[:, :, 0:128])
            nc.sync.dma_start(out=wka[:, :, 128:512], in_=wk_r[:, :, 128:512])
            nc.scalar.dma_start(out=cos2, in_=cos_d)
            nc.scalar.dma_start(out=sin2f, in_=sin_d)
            nc.scalar.dma_start(out=mask2, in_=mask_d)
            half_load(wva, wv_r, 0, nc.sync)
            half_load(wqa, wq_r, 1, nc.sync)
            half_load(wka, wk_r, 1, nc.sync)
            half_load(xTa, xT_r, 1, nc.sync)
            half_load(wva, wv_r, 1, nc.sync)
            nc.sync.dma_start(out=woa, in_=wo_r)

            # ---- persistent intermediates
            # qkrot[m][b]: [:, 0, :] = qrot, [:, 1, :] = krot (contiguous per batch)
            qkrot = [[persist.tile([128, 2, S], bf16, name=f"qkrot{m}_{b}")
                      for b in range(2)] for m in range(KC)]
            # v token-major, per head padded with a ones column (65 per head)
            vsb = [persist.tile([128, H * 65], bf16, name=f"vsb{t_}") for t_ in range(T // 128)]
            att = [persist.tile([128, T], bf16, name=f"att{m}") for m in range(KC)]

            for t_ in range(T // 128):
                vt = vsb[t_].rearrange("p (h w) -> p h w", w=65)
                nc.gpsimd.memset(vt[:, :, 64:65], 1.0)

            # dummy exp: pull the ACT exp table load into the DMA phase
            dumm = work.tile([1, 8], f32, name="dumm", tag="dumm", bufs=1)
            nc.gpsimd.memset(dumm, 0.0)
            nc.scalar.activation(dumm, dumm, Exp, scale=0.125)

            # ---- phase emitters
            def emit_qk_group(b, m):
                cols = slice(b * S, (b + 1) * S)
                pre2 = work.tile([128, 2, S], bf16, name="pre2", tag="pre2", bufs=2)
                for qk, w_sb in ((0, wq), (1, wk)):
                    pp = ps_p.tile([128, S], f32, name="pp", tag="ps_p")
                    for k in range(KC):
                        nc.tensor.matmul(
                            pp, w_sb[k][:, m * 128:(m + 1) * 128], xT[k][:, cols],
                            start=(k == 0), stop=(k == KC - 1))
                    nc.scalar.copy(pre2[:, qk, :], pp)   # ACT: psum -> sbuf bf16
                # rotate_half: 4 partition-block-swap DMAs (b0 on gpsimd ring,
                # b1 on the sync ring which is idle once loads finish)
                deng = nc.gpsimd if b == 0 else nc.sync
                prot2 = work.tile([128, 2, S], bf16, name="prot2", tag="prot2", bufs=2)
                for blk in range(4):
                    src = slice((blk ^ 1) * 32, (blk ^ 1) * 32 + 32)
                    dst = slice(blk * 32, blk * 32 + 32)
                    deng.dma_start(out=prot2[dst, :, :], in_=pre2[src, :, :])
                t1 = work.tile([128, 2, S], bf16, name="t1", tag="t1", bufs=2)
                nc.vector.tensor_mul(t1, pre2, cos2)
                t2 = work.tile([128, 2, S], bf16, name="t2", tag="t2", bufs=2)
                nc.vector.tensor_mul(t2, prot2, sin2f)
                nc.vector.tensor_add(qkrot[m][b], t1, t2)

            def emit_v_group(b, tch, nh):
                vt = vsb[tch].rearrange("p (h w) -> p h w", w=65)
                vp = ps_p.tile([128, S], f32, name="vp", tag="ps_p")
                for k in range(KC):
                    nc.tensor.matmul(
                        vp, xT[k][:, tch * 128:(tch + 1) * 128],
                        wv[k][:, nh * S:(nh + 1) * S],
                        start=(k == 0), stop=(k == KC - 1))
                nc.scalar.copy(
                    vt[:, nh * 8:(nh + 1) * 8, 0:64],
                    vp.rearrange("p (h w) -> p h w", w=64))

            # attention pair stages; state tiles kept per pair index
            stash = {}

            def attn_stage0(b, j):
                """scores -> exp -> mask -> attn@v -> evacuate -> st reshape"""
                mh = j
                exs = []
                for i in range(4):
                    lo = i * 128
                    sc = ps_sc.tile([128, 2, S], f32, name="sc", tag="ps_sc")
                    for hi, p0 in ((0, 0), (1, 64)):
                        nc.tensor.matmul(
                            sc[:, hi, 0:S - lo],
                            qkrot[mh][b][p0:p0 + 64, 1, lo:lo + 128],
                            qkrot[mh][b][p0:p0 + 64, 0, lo:S],
                            start=True, stop=True)
                    ex = expp.tile([128, 2, S], bf16, name="ex", tag=f"ex{i}")
                    nc.scalar.activation(ex[:, :, lo:S], sc[:, :, 0:S - lo], Exp, scale=0.125)
                    nc.vector.tensor_mul(ex[:, :, lo:lo + 128], ex[:, :, lo:lo + 128], mask2)
                    exs.append(ex)
                av = ps_av.tile([128, 2, S], f32, name="av", tag="ps_av")
                for hi in (0, 1):
                    h = 2 * j + hi
                    for i in range(4):
                        lo = i * 128
                        nc.tensor.matmul(
                            av[0:65, hi, lo:S],
                            vsb[b * 4 + i][:, h * 65: h * 65 + 65],
                            exs[i][:, hi, lo:S],
                            start=(i == 0), stop=(i == 3), skip_group_check=True)
                # single evacuation: raw att (rows 0:64) + sums row (64)
                attss = work.tile([65, 2, S], bf16, name="attss", tag="attss", bufs=3)
                nc.vector.tensor_copy(attss, av[0:65, :, :])
                # reshape sums row so the reciprocal uses all DVE lanes
                st = work.tile([128, 8], bf16, name="st", tag="st", bufs=2)
                nc.sync.dma_start(out=st, in_=attss[64:65, :, :])
                stash[(b, j)] = (attss, st)

            def attn_stage1(b, j):
                """reciprocal -> cast+reshape back -> broadcast to 64 rows"""
                attss, st = stash[(b, j)]
                rt = work.tile([128, 8], bf16, name="rt", tag="rt", bufs=2)
                with nc.allow_low_precision("softmax denominators in bf16"):
                    nc.vector.reciprocal(rt, st)
                rr = work.tile([1, 2 * S], bf16, name="rr", tag="rr", bufs=2)
                nc.gpsimd.dma_start(out=rr, in_=rt)
                rb = work.tile([64, 2 * S], bf16, name="rb", tag="rb", bufs=2)
                nc.gpsimd.partition_broadcast(rb, rr)
                stash[(b, j)] = (attss, rb)

            def attn_stage2(b, j):
                attss, rb = stash.pop((b, j))
                bcols = slice(b * S, (b + 1) * S)
                nc.vector.tensor_mul(att[j][0:64, bcols], attss[0:64, 0, :], rb[:, 0:S])
                nc.vector.tensor_mul(att[j][64:128, bcols], attss[0:64, 1, :], rb[:, S:2 * S])

            def emit_wo_group(b, m, ob_eng):
                bcols = slice(b * S, (b + 1) * S)
                fin = ps_p.tile([128, S], f32, name="fin", tag="ps_p")
                for k in range(KC):
                    nc.tensor.matmul(
                        fin, wo[k][:, m * 128:(m + 1) * 128], att[k][:, bcols],
                        start=(k == 0), stop=(k == KC - 1))
                ob = work.tile([128, S], bf16, name="ob", tag="ob", bufs=2)
                if ob_eng == "scalar":
                    nc.scalar.copy(ob, fin)
                else:
                    nc.vector.tensor_copy(ob, fin)
                nc.sync.dma_start(out=out_d[m * 128:(m + 1) * 128, bcols], in_=ob)

            # ---- emission schedule
            # phase A: b0 projections (ordered to chase the load stream)
            for m in range(4):
                emit_qk_group(0, m)
            for tch in range(4):
                emit_v_group(0, tch, 0)
            for m in range(4, KC):
                emit_qk_group(0, m)
            for tch in range(4):
                emit_v_group(0, tch, 1)
            # phase B: pairs b0 + staggered normalize; fills: qk b1, v b1.
            # NOTE: wo(b, m) reads ALL att chunks of batch b, so wo(0,*) may
            # only be emitted after stage2(0, 7), and wo(1,*) after stage2(1, 7).
            v1 = [(4, 0), (5, 0), (6, 0), (7, 0), (4, 1), (5, 1)]
            for j in range(8):
                attn_stage0(0, j)
                if j < 4:
                    emit_qk_group(1, j)
                if j in (6, 7):
                    emit_qk_group(1, j - 2)
                if j >= 2:
                    emit_v_group(1, *v1[j - 2])
                if j >= 1:
                    attn_stage1(0, j - 1)
                if j >= 2:
                    attn_stage2(0, j - 2)
            # phase B tail
            emit_v_group(1, 6, 1)
            emit_v_group(1, 7, 1)
            # phase C: pairs b1; fills: remaining qk b1, wo b0
            for j in range(8):
                if j < 2:
                    emit_qk_group(1, 6 + j)
                attn_stage0(1, j)
                if j == 0:
                    attn_stage1(0, 7)
                if j == 1:
                    attn_stage2(0, 6)
                if j == 2:
                    attn_stage2(0, 7)
                if j >= 1 and j < 7:
                    attn_stage1(1, j - 1)
                if j >= 2 and j < 7:
                    attn_stage2(1, j - 2)
                if j >= 3:
                    emit_wo_group(0, j - 3, "vector")
                if j == 6:
                    attn_stage1(1, 6)
                if j == 7:
                    attn_stage2(1, 5)
                    attn_stage2(1, 6)
                    attn_stage1(1, 7)
            for m in (5, 6, 7):   # fill the C->D drain with leftover wo(0)
                emit_wo_group(0, m, "scalar")
            attn_stage2(1, 7)
            # phase D: wo b1 (needs all b1 att chunks)
            for m in range(KC):
                emit_wo_group(1, m, "scalar")

            if dump_debug:
                for m in range(KC):
                    rows = slice(m * 128, (m + 1) * 128)
                    for b in range(2):
                        cl = slice(b * S, (b + 1) * S)
                        nc.sync.dma_start(out=qrot_d[rows, cl], in_=qkrot[m][b][:, 0, :])
                        nc.sync.dma_start(out=krot_d[rows, cl], in_=qkrot[m][b][:, 1, :])
                    nc.sync.dma_start(out=att_d[rows, :], in_=att[m])
                for t_ in range(T // 128):
                    nc.sync.dma_start(out=v_d[t_ * 128:(t_ + 1) * 128, :], in_=vsb[t_])

    nc.compile()
    return nc


def _get_nc():
    if "nc" not in _CACHE:
        _CACHE["nc"] = _build_bass()
    return _CACHE["nc"]


def make_in_maps(x, Wq, Wk, Wv, Wo):
    """Host-side shard + layout prep: one input dict per core."""
    cos2, sin2f, mask2 = _host_consts()
    shared = {
        "WqT": np.ascontiguousarray(Wq.T).astype(BF16),
        "WkT": np.ascontiguousarray(Wk.T).astype(BF16),
        "WvT": np.ascontiguousarray(Wv.T).astype(BF16),
        "WoT": np.ascontiguousarray(Wo.T).astype(BF16),
        "cos2": cos2,
        "sin2f": sin2f,
        "mask2": mask2,
    }
    in_maps = []
    for c in range(NCORES):
        xc = x[c * BPC:(c + 1) * BPC]  # [BPC, S, D]
        xT = np.ascontiguousarray(xc.transpose(2, 0, 1).reshape(D, T)).astype(BF16)
        in_maps.append({"xT": xT, **shared})
    return in_maps


def assemble(results):
    """results: list (per core) of {"outT": [D, T] bf16} -> [B, S, D] fp32."""
    out = np.empty((B, S, D), np.float32)
    for c in range(NCORES):
        oT = np.asarray(results[c]["outT"]).astype(np.float32)
        out[c * BPC:(c + 1) * BPC] = oT.reshape(D, BPC, S).transpose(1, 2, 0)
    return out


def run(x, Wq, Wk, Wv, Wo, trace=False, **run_kwargs):
    from concourse.bass_utils import run_bass_kernel_spmd
    nc = _get_nc()
    in_maps = make_in_maps(x, Wq, Wk, Wv, Wo)
    res = run_bass_kernel_spmd(
        nc, in_maps, core_ids=list(range(NCORES)), trace=trace, **run_kwargs)
    return assemble(res.results), res


def kernel(x, Wq, Wk, Wv, Wo):
    out, _ = run(np.asarray(x), np.asarray(Wq), np.asarray(Wk),
                 np.asarray(Wv), np.asarray(Wo))
    return out


if __name__ == "__main__":
    rng = np.random.default_rng(0)
    scale = 1.0 / np.sqrt(D)
    inputs = {
        "x": rng.standard_normal((B, S, D), dtype=np.float32),
        "Wq": (rng.standard_normal((D, D), dtype=np.float32) * scale),
        "Wk": (rng.standard_normal((D, D), dtype=np.float32) * scale),
        "Wv": (rng.standard_normal((D, D), dtype=np.float32) * scale),
        "Wo": (rng.standard_normal((D, D), dtype=np.float32) * scale),
    }
    out = kernel(**inputs)
    print("out", out.shape, out.dtype, float(np.abs(out).max()))
